# revision 4
# baseline (speedup 1.0000x reference)
"""BitConv2d inference kernel for Trainium2 (8 NeuronCores, SPMD).

Problem: y = conv2d(x, w_q.float(), stride=1, pad=1) * s + bias
  x:    (32, 128, 56, 56) f32
  w_q:  (256, 128, 3, 3) ternary {-1,0,+1} (int8 or int32)
  s:    (256, 1, 1) f32
  bias: (256,) f32
  y:    (32, 256, 56, 56) f32

Strategy: data-parallel over batch (4 images per core). On each core the
conv is 9 shifted matmuls per output tile: x is laid out channel-major
[C_in=128 partitions, (H+2)*(W+2)] with a zero border, so the rhs for
tap (kh, kw) is a contiguous slice. Output tiles are [128 C_out-chunk,
8 rows * 58] PSUM accumulations; scale+bias applied on ScalarE while
slicing off the 2 pad columns per row; dense DMA out.

x is fed in fp16 (hi) with an optional exact residual pass: lo =
(x - fp16(x)) * 2^11 in fp16 against weights pre-scaled by 2^-11,
accumulated into the same PSUM group -> near-fp32 accuracy.
"""

import os

import numpy as np

import concourse.bass as bass
import concourse.mybir as mybir
from concourse import bacc
from concourse.tile import TileContext

# Problem constants (hardcoded per contract)
N_IMG, C_IN, C_OUT, H, W = 32, 128, 256, 56, 56
N_CORES = 8
IMG_PER_CORE = N_IMG // N_CORES  # 4
HP, WP = H + 2, W + 2  # 58, 58
IMG_ELEMS = HP * WP  # 3364
# per-image SBUF columns incl. slack for the tail tap overshoot
IMG_COLS = IMG_ELEMS + 4  # 3368
ROWS_PER_BLK = 8
N_BLK = H // ROWS_PER_BLK  # 7
FREE = ROWS_PER_BLK * WP  # 464  (<= 512 fp32 PSUM bank)
OUT_FREE = ROWS_PER_BLK * W  # 448
N_CHUNK = C_OUT // 128  # 2
LO_SCALE = 2048.0  # 2^11, exact in fp16

# PASSES: 1 = fp16 hi only (~3e-4 rel err), 2 = hi + exact residual (~1e-7)
PASSES = int(os.environ.get("BITCONV_PASSES", "2"))

f16 = mybir.dt.float16
f32 = mybir.dt.float32


def build_nc(passes: int) -> bacc.Bacc:
    nc = bacc.Bacc("TRN2", target_bir_lowering=False, debug=False)

    x_in = [
        nc.dram_tensor(f"x{p}", [IMG_PER_CORE, C_IN, IMG_COLS], f16,
                       kind="ExternalInput").ap()
        for p in range(passes)
    ]
    n_wblk = passes * N_CHUNK * 9
    wt = nc.dram_tensor("wt", [C_IN, n_wblk * 128], f16, kind="ExternalInput").ap()
    sv = nc.dram_tensor("sv", [128, N_CHUNK], f32, kind="ExternalInput").ap()
    bv = nc.dram_tensor("bv", [128, N_CHUNK], f32, kind="ExternalInput").ap()
    y = nc.dram_tensor("y", [IMG_PER_CORE, C_OUT, H, W], f32,
                       kind="ExternalOutput").ap()

    with TileContext(nc) as tc:
        with (
            tc.tile_pool(name="xpool", bufs=IMG_PER_CORE * passes) as xpool,
            tc.tile_pool(name="wpool", bufs=1) as wpool,
            tc.tile_pool(name="cpool", bufs=1) as cpool,
            tc.tile_pool(name="opool", bufs=6) as opool,
            tc.tile_pool(name="ppool", bufs=6, space="PSUM") as ppool,
        ):
            wt_t = wpool.tile([C_IN, n_wblk * 128], f16)
            nc.sync.dma_start(out=wt_t[:, :], in_=wt[:, :])
            sv_t = cpool.tile([128, N_CHUNK], f32, tag="sv")
            bv_t = cpool.tile([128, N_CHUNK], f32, tag="bv")
            nc.sync.dma_start(out=sv_t[:, :], in_=sv[:, :])
            nc.sync.dma_start(out=bv_t[:, :], in_=bv[:, :])

            xt = [[None] * IMG_PER_CORE for _ in range(passes)]
            for i in range(IMG_PER_CORE):
                for p in range(passes):
                    t = xpool.tile([C_IN, IMG_COLS], f16, tag="ximg")
                    nc.sync.dma_start(out=t[:, :], in_=x_in[p][i])
                    xt[p][i] = t

            nmm = passes * 9
            for i in range(IMG_PER_CORE):
                for b in range(N_BLK):
                    for c in range(N_CHUNK):
                        ps = ppool.tile([128, FREE], f32, tag="ps")
                        k = 0
                        for p in range(passes):
                            for kh in range(3):
                                for kw in range(3):
                                    off = (b * ROWS_PER_BLK + kh) * WP + kw
                                    blk = (p * N_CHUNK + c) * 9 + kh * 3 + kw
                                    nc.tensor.matmul(
                                        ps[:, :],
                                        wt_t[:, blk * 128:(blk + 1) * 128],
                                        xt[p][i][:, off:off + FREE],
                                        start=(k == 0),
                                        stop=(k == nmm - 1),
                                    )
                                    k += 1
                        ot = opool.tile([128, OUT_FREE], f32, tag="ot")
                        ps3 = ps[:, :].rearrange(
                            "q (r c) -> q r c", r=ROWS_PER_BLK)[:, :, 0:W]
                        ot3 = ot[:, :].rearrange(
                            "q (r c) -> q r c", r=ROWS_PER_BLK)
                        nc.scalar.activation(
                            ot3, ps3, mybir.ActivationFunctionType.Identity,
                            bias=bv_t[:, c:c + 1], scale=sv_t[:, c:c + 1])
                        nc.sync.dma_start(
                            out=y[i, c * 128:(c + 1) * 128,
                                  b * ROWS_PER_BLK:(b + 1) * ROWS_PER_BLK, :],
                            in_=ot3)

    nc.compile()
    return nc


def prep_inputs(x, w_q, s, bias, passes: int):
    """Full inputs -> list of 8 per-core in_maps (numpy)."""
    x = np.asarray(x, dtype=np.float32)
    wq = np.asarray(w_q).astype(np.float32)
    s = np.asarray(s, dtype=np.float32).reshape(C_OUT)
    bias = np.asarray(bias, dtype=np.float32).reshape(C_OUT)

    # x -> fp16 hi (+ scaled fp16 residual), padded, channel-major
    x_hi = x.astype(np.float16)
    parts = [x_hi]
    if passes == 2:
        x_lo = ((x - x_hi.astype(np.float32)) * LO_SCALE).astype(np.float16)
        parts.append(x_lo)

    in_maps = [dict() for _ in range(N_CORES)]
    for p, xp in enumerate(parts):
        pad = np.zeros((N_CORES, IMG_PER_CORE, C_IN, HP, WP), np.float16)
        # interior [1:57, 1:57] of each 58x58 image; border stays zero
        pad[:, :, :, 1:H + 1, 1:W + 1] = xp.reshape(
            N_CORES, IMG_PER_CORE, C_IN, H, W)
        buf = np.zeros((N_CORES, IMG_PER_CORE, C_IN, IMG_COLS), np.float16)
        buf[:, :, :, :IMG_ELEMS] = pad.reshape(
            N_CORES, IMG_PER_CORE, C_IN, IMG_ELEMS)
        for core in range(N_CORES):
            in_maps[core][f"x{p}"] = buf[core]

    # weights: wt[p_cin, blk, m] = w_q[c*128+m, p_cin, kh, kw] (* lo scale)
    w5 = np.transpose(wq.reshape(N_CHUNK, 128, C_IN, 3, 3), (2, 0, 3, 4, 1))
    w5 = np.ascontiguousarray(w5).reshape(C_IN, N_CHUNK * 9 * 128)
    blocks = [w5]
    if passes == 2:
        blocks.append(w5 * (1.0 / LO_SCALE))
    wt = np.concatenate(blocks, axis=1).astype(np.float16)

    sv = np.ascontiguousarray(s.reshape(N_CHUNK, 128).T)
    bv = np.ascontiguousarray(bias.reshape(N_CHUNK, 128).T)
    for core in range(N_CORES):
        in_maps[core]["wt"] = wt
        in_maps[core]["sv"] = sv
        in_maps[core]["bv"] = bv
    return in_maps


_NC_CACHE: dict[int, bacc.Bacc] = {}


def get_nc(passes: int) -> bacc.Bacc:
    if passes not in _NC_CACHE:
        _NC_CACHE[passes] = build_nc(passes)
    return _NC_CACHE[passes]


def run(inputs, trace: bool = False, passes: int = PASSES, **run_kwargs):
    """Returns (full_output, BassKernelResults)."""
    from concourse.bass_utils import run_bass_kernel_spmd

    nc = get_nc(passes)
    in_maps = prep_inputs(**inputs, passes=passes)
    res = run_bass_kernel_spmd(nc, in_maps, list(range(N_CORES)),
                               trace=trace, **run_kwargs)
    out = np.concatenate([np.asarray(res.results[i]["y"])
                          for i in range(N_CORES)], axis=0)
    return out, res


def kernel(**inputs) -> np.ndarray:
    out, _ = run(inputs)
    return out


# revision 6
# speedup vs baseline: 1.0530x; 1.0530x over previous
"""BitConv2d inference kernel for Trainium2 (8 NeuronCores, SPMD).

Problem: y = conv2d(x, w_q.float(), stride=1, pad=1) * s + bias
  x:    (32, 128, 56, 56) f32
  w_q:  (256, 128, 3, 3) ternary {-1,0,+1} (int8 or int32)
  s:    (256, 1, 1) f32
  bias: (256,) f32
  y:    (32, 256, 56, 56) f32

Strategy: data-parallel over batch (4 images per core). On each core the
conv is 9 shifted matmuls per output tile: x is laid out channel-major
[C_in=128 partitions, flat padded image] with row stride 57 (the single
zero column between consecutive rows serves as both right-pad of row r
and left-pad of row r+1), so the rhs for tap (kh, kw) is a contiguous
slice. Output tiles are [128 C_out-chunk, 8 rows * 57] PSUM
accumulations; scale+bias applied on ScalarE while dropping the pad
column per row; dense DMA out.

x is fed in fp16 (hi) with an optional exact residual pass: lo =
(x - fp16(x)) * 2^11 in fp16 against weights pre-scaled by 2^-11,
accumulated into the same PSUM group -> near-fp32 accuracy.

Each image is split into a top chunk (output blocks 0-3) and a bottom
chunk (blocks 4-6) so the first matmuls only wait for ~1MB of DMA, and
dummy warm-up matmuls keep the PE busy during that wait (HAM un-throttle
to 2.4 GHz costs ~3.4us of sustained activity).
"""

import os

import numpy as np

import concourse.bass as bass
import concourse.mybir as mybir
from concourse import bacc
from concourse.tile import TileContext

# Problem constants (hardcoded per contract)
N_IMG, C_IN, C_OUT, H, W = 32, 128, 256, 56, 56
N_CORES = 8
IMG_PER_CORE = N_IMG // N_CORES  # 4
S = W + 1  # 57: flat row stride; col 56 of row r == left pad of row r+1
ROWS_PER_BLK = 8
N_BLK = H // ROWS_PER_BLK  # 7
FREE = ROWS_PER_BLK * S  # 456  (<= 512 fp32 PSUM bank)
OUT_FREE = ROWS_PER_BLK * W  # 448
N_CHUNK = C_OUT // 128  # 2
LO_SCALE = 2048.0  # 2^11, exact in fp16

# padded flat image P[k], k = r*57 + c, r in 0..57 (58 rows: top/bottom pad)
# P[r*57+c] = x[r-1, c-1] for r,c in 1..56; P[r*57] = 0; + slack for the
# tail tap overshoot (block b, tap kh,kw reads [(b*8+kh)*57+kw : +456]).
P_ELEMS = 58 * S + 1  # 3307
TOP_BLKS = 4  # output blocks 0..3 read padded rows 0..33
TOP_ROWS = TOP_BLKS * ROWS_PER_BLK + 2  # 34
TOP_COLS = TOP_ROWS * S + 2  # 1940 (covers (26)*57+2+456)
BOT_ROW0 = TOP_BLKS * ROWS_PER_BLK  # padded row 32
BOT_COLS = P_ELEMS - BOT_ROW0 * S + 1  # 1484 (covers (18)*57+2+456)

# PASSES: 1 = fp16 hi only (~2e-4 rel err), 2 = hi + exact residual (~3e-7)
PASSES = int(os.environ.get("BITCONV_PASSES", "1"))
N_WARMUP = int(os.environ.get("BITCONV_WARMUP", "18"))
WARMUP_FREE = 256

f16 = mybir.dt.float16
f32 = mybir.dt.float32


def build_nc(passes: int) -> bacc.Bacc:
    nc = bacc.Bacc("TRN2", target_bir_lowering=False, debug=False)

    xtop = [
        nc.dram_tensor(f"xt{p}", [IMG_PER_CORE, C_IN, TOP_COLS], f16,
                       kind="ExternalInput").ap()
        for p in range(passes)
    ]
    xbot = [
        nc.dram_tensor(f"xb{p}", [IMG_PER_CORE, C_IN, BOT_COLS], f16,
                       kind="ExternalInput").ap()
        for p in range(passes)
    ]
    n_wblk = passes * N_CHUNK * 9
    wt = nc.dram_tensor("wt", [C_IN, n_wblk * 128], f16, kind="ExternalInput").ap()
    sv = nc.dram_tensor("sv", [128, N_CHUNK], f32, kind="ExternalInput").ap()
    bv = nc.dram_tensor("bv", [128, N_CHUNK], f32, kind="ExternalInput").ap()
    y = nc.dram_tensor("y", [IMG_PER_CORE, C_OUT, H, W], f32,
                       kind="ExternalOutput").ap()

    with TileContext(nc) as tc:
        with (
            tc.tile_pool(name="xpool", bufs=IMG_PER_CORE * passes) as xpool,
            tc.tile_pool(name="wpool", bufs=1) as wpool,
            tc.tile_pool(name="cpool", bufs=1) as cpool,
            tc.tile_pool(name="opool", bufs=6) as opool,
            tc.tile_pool(name="ppool", bufs=6, space="PSUM") as ppool,
            tc.tile_pool(name="wps", bufs=1, space="PSUM") as wps_pool,
        ):
            # scratch for PE warm-up (zeros; written before first DMA lands)
            wu = cpool.tile([128, WARMUP_FREE], f16, tag="wu")
            nc.vector.memset(wu[:, :], 0.0)
            wu_ps = wps_pool.tile([128, WARMUP_FREE], f32, tag="wups")

            # weights + first image chunks first: they gate the first matmul
            wt_t = wpool.tile([C_IN, n_wblk * 128], f16)
            nc.sync.dma_start(out=wt_t[:, :], in_=wt[:, :])

            xt_t = [[None] * IMG_PER_CORE for _ in range(passes)]
            xb_t = [[None] * IMG_PER_CORE for _ in range(passes)]
            for i in range(IMG_PER_CORE):
                for p in range(passes):
                    t = xpool.tile([C_IN, TOP_COLS], f16, tag="xtop")
                    nc.sync.dma_start(out=t[:, :], in_=xtop[p][i])
                    xt_t[p][i] = t
                    b = xpool.tile([C_IN, BOT_COLS], f16, tag="xbot")
                    nc.sync.dma_start(out=b[:, :], in_=xbot[p][i])
                    xb_t[p][i] = b
                if i == 0:
                    sv_t = cpool.tile([128, N_CHUNK], f32, tag="sv")
                    bv_t = cpool.tile([128, N_CHUNK], f32, tag="bv")
                    nc.sync.dma_start(out=sv_t[:, :], in_=sv[:, :])
                    nc.sync.dma_start(out=bv_t[:, :], in_=bv[:, :])

            # HAM warm-up: dummy matmuls on the zero scratch keep the PE
            # active while the first real DMAs are in flight.
            for _ in range(N_WARMUP):
                nc.tensor.matmul(wu_ps[:, :], wu[:, 0:128], wu[:, :],
                                 start=True, stop=True)

            nmm = passes * 9
            for i in range(IMG_PER_CORE):
                for b in range(N_BLK):
                    top = b < TOP_BLKS
                    row0 = b * ROWS_PER_BLK - (0 if top else BOT_ROW0)
                    for c in range(N_CHUNK):
                        ps = ppool.tile([128, FREE], f32, tag="ps")
                        k = 0
                        for p in range(passes):
                            src = (xt_t if top else xb_t)[p][i]
                            for kh in range(3):
                                for kw in range(3):
                                    off = (row0 + kh) * S + kw
                                    blk = (p * N_CHUNK + c) * 9 + kh * 3 + kw
                                    nc.tensor.matmul(
                                        ps[:, :],
                                        wt_t[:, blk * 128:(blk + 1) * 128],
                                        src[:, off:off + FREE],
                                        start=(k == 0),
                                        stop=(k == nmm - 1),
                                    )
                                    k += 1
                        ot = opool.tile([128, OUT_FREE], f32, tag="ot")
                        ps3 = ps[:, :].rearrange(
                            "q (r c) -> q r c", r=ROWS_PER_BLK)[:, :, 0:W]
                        ot3 = ot[:, :].rearrange(
                            "q (r c) -> q r c", r=ROWS_PER_BLK)
                        nc.scalar.activation(
                            ot3, ps3, mybir.ActivationFunctionType.Identity,
                            bias=bv_t[:, c:c + 1], scale=sv_t[:, c:c + 1])
                        nc.sync.dma_start(
                            out=y[i, c * 128:(c + 1) * 128,
                                  b * ROWS_PER_BLK:(b + 1) * ROWS_PER_BLK, :],
                            in_=ot3)

    nc.compile()
    return nc


def prep_inputs(x, w_q, s, bias, passes: int):
    """Full inputs -> list of 8 per-core in_maps (numpy)."""
    x = np.asarray(x, dtype=np.float32)
    wq = np.asarray(w_q).astype(np.float32)
    s = np.asarray(s, dtype=np.float32).reshape(C_OUT)
    bias = np.asarray(bias, dtype=np.float32).reshape(C_OUT)

    # x -> fp16 hi (+ scaled fp16 residual)
    x_hi = x.astype(np.float16)
    parts = [x_hi]
    if passes == 2:
        x_lo = ((x - x_hi.astype(np.float32)) * LO_SCALE).astype(np.float16)
        parts.append(x_lo)

    in_maps = [dict() for _ in range(N_CORES)]
    for p, xp in enumerate(parts):
        # padded flat layout P: 58 rows of stride 57 (+ tail slack)
        buf = np.zeros((N_CORES, IMG_PER_CORE, C_IN, P_ELEMS + 3), np.float16)
        v = np.lib.stride_tricks.as_strided(
            buf[:, :, :, S + 1:],  # row r=1, col c=1
            shape=(N_CORES, IMG_PER_CORE, C_IN, H, W),
            strides=buf.strides[:3] + (buf.strides[3] * S, buf.strides[3]),
        )
        v[:] = xp.reshape(N_CORES, IMG_PER_CORE, C_IN, H, W)
        for core in range(N_CORES):
            in_maps[core][f"xt{p}"] = np.ascontiguousarray(
                buf[core, :, :, :TOP_COLS])
            in_maps[core][f"xb{p}"] = np.ascontiguousarray(
                buf[core, :, :, BOT_ROW0 * S:BOT_ROW0 * S + BOT_COLS])

    # weights: wt[p_cin, blk, m] = w_q[c*128+m, p_cin, kh, kw] (* lo scale)
    w5 = np.transpose(wq.reshape(N_CHUNK, 128, C_IN, 3, 3), (2, 0, 3, 4, 1))
    w5 = np.ascontiguousarray(w5).reshape(C_IN, N_CHUNK * 9 * 128)
    blocks = [w5]
    if passes == 2:
        blocks.append(w5 * (1.0 / LO_SCALE))
    wt = np.concatenate(blocks, axis=1).astype(np.float16)

    sv = np.ascontiguousarray(s.reshape(N_CHUNK, 128).T)
    bv = np.ascontiguousarray(bias.reshape(N_CHUNK, 128).T)
    for core in range(N_CORES):
        in_maps[core]["wt"] = wt
        in_maps[core]["sv"] = sv
        in_maps[core]["bv"] = bv
    return in_maps


_NC_CACHE: dict[int, bacc.Bacc] = {}


def get_nc(passes: int) -> bacc.Bacc:
    if passes not in _NC_CACHE:
        _NC_CACHE[passes] = build_nc(passes)
    return _NC_CACHE[passes]


def run(inputs, trace: bool = False, passes: int = PASSES, **run_kwargs):
    """Returns (full_output, BassKernelResults)."""
    from concourse.bass_utils import run_bass_kernel_spmd

    nc = get_nc(passes)
    in_maps = prep_inputs(**inputs, passes=passes)
    res = run_bass_kernel_spmd(nc, in_maps, list(range(N_CORES)),
                               trace=trace, **run_kwargs)
    out = np.concatenate([np.asarray(res.results[i]["y"])
                          for i in range(N_CORES)], axis=0)
    return out, res


def kernel(**inputs) -> np.ndarray:
    out, _ = run(inputs)
    return out
